# revision 5
# baseline (speedup 1.0000x reference)
"""Multi-head causal attention (B=8, S=1024, E=512, H=8, Dk=Dv=64) on 8 NeuronCores.

Sharding: data-parallel over batch. Core b computes the full attention block
for X[b]; no collectives. Host pre-transposes X[b] -> [E, S], converts matmul
operands to bf16, and pre-arranges weights so the device kernel is pure
matmul + softmax.

Per-core dataflow (bf16 matmuls, fp32 PSUM accumulate / softmax math):
  XT [E,S] resident in SBUF (4 tiles of [128, 1024])
  V  = (X @ Wv + bv)            -> 8 tiles [128 s, 512 hd]
  QT = (X @ Wq + bq)^T          -> per head-pair tiles [128 dd, 512 q] (x2 q-halves)
  KT likewise
  per head h, q-chunk qc (512 cols):
    scores^T blocks [128 k, 512 q] (k-blocks above the diagonal skipped),
    causal mask added from a precomputed staircase slab, exp on ScalarE
    (scale=1/8 folded in) over two-block PSUM groups,
    O^T accum = V-slice^T @ exp-blocks,  denom = ones^T @ exp-blocks
    (ones lhsT is M=128 so the denominator row broadcasts),
    O^T *= 1/denom via fast-NR reciprocal.
  Y[s-chunk] = sum_h O_h^T-block^T @ Wo_h + bo
"""

import numpy as np
import ml_dtypes

import concourse.bass as bass
import concourse.tile as tile
import concourse.mybir as mybir
from concourse import bacc
from concourse import bass_utils

B, S, E = 8, 1024, 512
H, DK, DV = 8, 64, 64
HD = H * DK  # 512
P = 128
EC = E // P  # 4 contraction chunks over E
NPAIR = H // 2
QCN = S // 512  # 2 q-chunks of 512
NCORES = 8
F32 = mybir.dt.float32
BF16 = mybir.dt.bfloat16
NEG = -1.0e9

_COMPILED = None


def _body(nc, tc, const, work, ps, pb, d):
    # ---- load constants / inputs into SBUF ----
    xt = []
    for c in range(EC):
        t = const.tile([P, S], BF16, tag=f"xt{c}", name=f"xt{c}")
        nc.sync.dma_start(t[:], d["xt"][c * P:(c + 1) * P, :])
        xt.append(t)
    w_sb = {}
    for wname in ("wq", "wk", "wv"):
        tiles = []
        for c in range(EC):
            t = const.tile([P, HD], BF16, tag=f"{wname}{c}", name=f"{wname}{c}")
            nc.sync.dma_start(t[:], d[wname][c * P:(c + 1) * P, :])
            tiles.append(t)
        w_sb[wname] = tiles
    woh = []
    for h in range(H):
        t = const.tile([DV, E], BF16, tag=f"woh{h}", name=f"woh{h}")
        nc.sync.dma_start(t[:], d["woh"][h])
        woh.append(t)
    mask_t = const.tile([P, 512], F32, tag="mask", name="mask_t")
    nc.sync.dma_start(mask_t[:], d["mask"][:])
    ones_t = const.tile([P, P], BF16, tag="ones", name="ones_t")
    nc.sync.dma_start(ones_t[:], d["ones"][:])
    bq_t = const.tile([P, NPAIR], F32, tag="bq", name="bq_t")
    nc.sync.dma_start(bq_t[:], d["bq"][:])
    bk_t = const.tile([P, NPAIR], F32, tag="bk", name="bk_t")
    nc.sync.dma_start(bk_t[:], d["bk"][:])
    bvb_t = const.tile([P, HD], F32, tag="bvb", name="bvb_t")
    nc.sync.dma_start(bvb_t[:], d["bvb"][:])
    bob_t = const.tile([P, E], F32, tag="bob", name="bob_t")
    nc.sync.dma_start(bob_t[:], d["bob"][:])

    # ---- V = X @ Wv + bv : per s-chunk [128 s, 512 hd] ----
    v_sb = []
    for si in range(S // P):
        vp = ps.tile([P, HD], F32, tag="ps512", name=f"vp{si}")
        for c in range(EC):
            nc.tensor.matmul(
                vp[:], xt[c][:, si * P:(si + 1) * P], w_sb["wv"][c][:],
                start=(c == 0), stop=(c == EC - 1))
        t = const.tile([P, HD], BF16, tag=f"v{si}", name=f"v{si}")
        nc.any.tensor_add(t[:], vp[:], bvb_t[:])
        v_sb.append(t)

    # ---- QT / KT per head-pair, per q-half: [128 dd, 512 s] ----
    qt = {}
    kt = {}
    for p in range(NPAIR):
        for qc in range(QCN):
            qp = ps.tile([P, 512], F32, tag="ps512", name=f"qtp{p}_{qc}")
            for c in range(EC):
                nc.tensor.matmul(
                    qp[:], w_sb["wq"][c][:, p * P:(p + 1) * P],
                    xt[c][:, qc * 512:(qc + 1) * 512],
                    start=(c == 0), stop=(c == EC - 1))
            t = const.tile([P, 512], BF16, tag=f"qt{p}_{qc}", name=f"qt{p}_{qc}")
            nc.any.tensor_scalar_add(t[:], qp[:], bq_t[:, p:p + 1])
            qt[p, qc] = t

            kp = ps.tile([P, 512], F32, tag="ps512", name=f"ktp{p}_{qc}")
            for c in range(EC):
                nc.tensor.matmul(
                    kp[:], w_sb["wk"][c][:, p * P:(p + 1) * P],
                    xt[c][:, qc * 512:(qc + 1) * 512],
                    start=(c == 0), stop=(c == EC - 1))
            t = const.tile([P, 512], BF16, tag=f"kt{p}_{qc}", name=f"kt{p}_{qc}")
            nc.any.tensor_scalar_add(t[:], kp[:], bk_t[:, p:p + 1])
            kt[p, qc] = t

    # ---- attention per head, per q-chunk ----
    ot_sb = {}
    for h in range(H):
        p, hb = h // 2, h % 2
        hp = slice(hb * DK, (hb + 1) * DK)  # head's rows within pair tiles
        for qc in range(QCN):
            n_ki = 4 * (qc + 1)  # causal: only k-blocks with ki*128 <= qc*512+511
            otp = ps.tile([DV, 512], F32, tag="ps512", name=f"otp{h}_{qc}")
            smp = ps.tile([P, 512], F32, tag="ps512", name=f"smp{h}_{qc}")
            for g in range(n_ki // 2):
                stp = pb.tile([P, 1024], F32, tag="st", name=f"st{h}_{qc}_{g}")
                for j in range(2):
                    ki = 2 * g + j
                    kc, kl = ki // 4, ki % 4
                    nc.tensor.matmul(
                        stp[:, j * 512:(j + 1) * 512],
                        kt[p, kc][hp, kl * P:(kl + 1) * P],
                        qt[p, qc][hp, :],
                        start=True, stop=True)
                    off = ki * P - qc * 512
                    if off >= 0:
                        w = off + P  # mask region always ends at column 512
                        nc.any.tensor_add(
                            stp[:, j * 512:j * 512 + w],
                            stp[:, j * 512:j * 512 + w],
                            mask_t[:, 512 - w:512])
                ste = work.tile([P, 1024], BF16, tag="ste", name=f"ste{h}_{qc}_{g}")
                nc.scalar.activation(
                    ste[:], stp[:], mybir.ActivationFunctionType.Exp, scale=0.125)
                for j in range(2):
                    ki = 2 * g + j
                    nc.tensor.matmul(
                        otp[:], v_sb[ki][:, h * DV:(h + 1) * DV],
                        ste[:, j * 512:(j + 1) * 512],
                        start=(ki == 0), stop=(ki == n_ki - 1))
                    nc.tensor.matmul(
                        smp[:], ones_t[:],
                        ste[:, j * 512:(j + 1) * 512],
                        start=(ki == 0), stop=(ki == n_ki - 1))
            rec = work.tile([DV, 512], F32, tag="rec", name=f"rec{h}_{qc}", bufs=2)
            nc.vector.reciprocal_approx_fast(rec[:], smp[0:DV, :])
            ot = const.tile([DV, 512], BF16, tag=f"ot{h}_{qc}", name=f"ot{h}_{qc}")
            nc.vector.tensor_mul(ot[:], otp[:], rec[:])
            ot_sb[h, qc] = ot

    # ---- output projection Y[s-chunk] = sum_h OT_h-block^T @ Wo_h + bo ----
    for si in range(S // P):
        qc, sl = si // 4, si % 4
        yp = ps.tile([P, E], F32, tag="ps512", name=f"yp{si}")
        for h in range(H):
            nc.tensor.matmul(
                yp[:], ot_sb[h, qc][:, sl * P:(sl + 1) * P], woh[h][:],
                start=(h == 0), stop=(h == H - 1))
        yo = work.tile([P, E], F32, tag="yo", name=f"yo{si}", bufs=2)
        nc.any.tensor_add(yo[:], yp[:], bob_t[:])
        nc.sync.dma_start(d["y"][si * P:(si + 1) * P, :], yo[:])


def _build():
    nc = bacc.Bacc("TRN2", target_bir_lowering=False, debug=False)
    d = {
        "xt": nc.dram_tensor("xt", [E, S], BF16, kind="ExternalInput").ap(),
        "wq": nc.dram_tensor("wq", [E, HD], BF16, kind="ExternalInput").ap(),
        "wk": nc.dram_tensor("wk", [E, HD], BF16, kind="ExternalInput").ap(),
        "wv": nc.dram_tensor("wv", [E, HD], BF16, kind="ExternalInput").ap(),
        "woh": nc.dram_tensor("woh", [H, DV, E], BF16, kind="ExternalInput").ap(),
        "mask": nc.dram_tensor("mask", [P, 512], F32, kind="ExternalInput").ap(),
        "ones": nc.dram_tensor("ones", [P, P], BF16, kind="ExternalInput").ap(),
        "bq": nc.dram_tensor("bq", [P, NPAIR], F32, kind="ExternalInput").ap(),
        "bk": nc.dram_tensor("bk", [P, NPAIR], F32, kind="ExternalInput").ap(),
        "bvb": nc.dram_tensor("bvb", [P, HD], F32, kind="ExternalInput").ap(),
        "bob": nc.dram_tensor("bob", [P, E], F32, kind="ExternalInput").ap(),
        "y": nc.dram_tensor("y", [S, E], F32, kind="ExternalOutput").ap(),
    }
    with tile.TileContext(nc) as tc:
        with tc.tile_pool(name="const", bufs=1) as const, \
             tc.tile_pool(name="work", bufs=3) as work, \
             tc.tile_pool(name="ps", bufs=4, space="PSUM") as ps, \
             tc.tile_pool(name="pb", bufs=2, space="PSUM") as pb:
            _body(nc, tc, const, work, ps, pb, d)
    nc.compile()
    return nc


def get_nc():
    global _COMPILED
    if _COMPILED is None:
        _COMPILED = _build()
    return _COMPILED


def _prep_in_maps(X, Wq, bq, Wk, bk, Wv, bv, Wo, bo):
    f = np.float32
    bf = ml_dtypes.bfloat16
    shared = {
        "wq": np.ascontiguousarray(
            np.transpose(np.asarray(Wq, f), (1, 0, 2)).reshape(E, HD).astype(bf)),
        "wk": np.ascontiguousarray(
            np.transpose(np.asarray(Wk, f), (1, 0, 2)).reshape(E, HD).astype(bf)),
        "wv": np.ascontiguousarray(
            np.transpose(np.asarray(Wv, f), (1, 0, 2)).reshape(E, HD).astype(bf)),
        "woh": np.ascontiguousarray(np.asarray(Wo, f).reshape(H, DV, E).astype(bf)),
        "ones": np.ones((P, P), bf),
        "bq": np.ascontiguousarray(np.asarray(bq, f).reshape(HD).reshape(NPAIR, P).T),
        "bk": np.ascontiguousarray(np.asarray(bk, f).reshape(HD).reshape(NPAIR, P).T),
        "bvb": np.ascontiguousarray(np.broadcast_to(np.asarray(bv, f).reshape(1, HD), (P, HD))),
        "bob": np.ascontiguousarray(np.broadcast_to(np.asarray(bo, f).reshape(1, E), (P, E))),
    }
    # staircase causal mask slab: M[k, j] = NEG where k > j - 384 (j in [0,512)).
    # block (ki, qc) with off = ki*128 - qc*512 >= 0 uses columns [512-w, 512).
    kk = np.arange(P)[:, None]
    jj = np.arange(512)[None, :]
    shared["mask"] = np.where(kk > jj - 384, f(NEG), f(0.0)).astype(f)
    Xf = np.asarray(X, f)
    in_maps = []
    for b in range(B):
        m = dict(shared)
        m["xt"] = np.ascontiguousarray(Xf[b].T.astype(bf))
        in_maps.append(m)
    return in_maps


def kernel(X, Wq, bq, Wk, bk, Wv, bv, Wo, bo):
    nc = get_nc()
    in_maps = _prep_in_maps(X, Wq, bq, Wk, bk, Wv, bv, Wo, bo)
    res = bass_utils.run_bass_kernel_spmd(nc, in_maps, core_ids=list(range(NCORES)))
    return np.stack([res.results[b]["y"] for b in range(B)], axis=0).astype(np.float32)


def run_traced(X, Wq, bq, Wk, bk, Wv, bv, Wo, bo):
    """Like kernel() but with NTFF profiling; returns (out, exec_time_ns)."""
    nc = get_nc()
    in_maps = _prep_in_maps(X, Wq, bq, Wk, bk, Wv, bv, Wo, bo)
    res = bass_utils.run_bass_kernel_spmd(
        nc, in_maps, core_ids=list(range(NCORES)), trace=True)
    out = np.stack([res.results[b]["y"] for b in range(B)], axis=0).astype(np.float32)
    return out, res.exec_time_ns


# revision 8
# speedup vs baseline: 1.6937x; 1.6937x over previous
"""Multi-head causal attention (B=8, S=1024, E=512, H=8, Dk=Dv=64) on 8 NeuronCores.

Sharding: data-parallel over batch. Core b computes the full attention block
for X[b]; no collectives. Host pre-transposes X[b] -> [E, S], converts matmul
operands to bf16, and pre-arranges weights so the device kernel is pure
matmul + softmax.

Per-core dataflow (bf16 matmuls, fp32 PSUM accumulate / softmax math):
  XT [E,S] resident in SBUF (4 tiles of [128, 1024])
  V  = (X @ Wv + bv)            -> 8 tiles [128 s, 512 hd]
  QT = (X @ Wq + bq)^T          -> per head-pair tiles [128 dd, 512 q] (x2 q-halves)
  KT likewise
  per head h, q-chunk qc (512 cols):
    scores^T blocks [128 k, 512 q] (k-blocks above the diagonal skipped),
    causal mask added from a precomputed staircase slab, exp on ScalarE
    (scale=1/8 folded in) over two-block PSUM groups,
    O^T accum = V-slice^T @ exp-blocks,  denom = ones^T @ exp-blocks
    (ones lhsT is M=128 so the denominator row broadcasts),
    O^T *= 1/denom via fast-NR reciprocal.
  Y[s-chunk] = sum_h O_h^T-block^T @ Wo_h + bo
"""

import numpy as np
import ml_dtypes

import concourse.bass as bass
import concourse.tile as tile
import concourse.mybir as mybir
from concourse import bacc
from concourse import bass_utils

B, S, E = 8, 1024, 512
H, DK, DV = 8, 64, 64
HD = H * DK  # 512
P = 128
EC = E // P  # 4 contraction chunks over E
NPAIR = H // 2
QCN = S // 512  # 2 q-chunks of 512
NCORES = 8
F32 = mybir.dt.float32
BF16 = mybir.dt.bfloat16
NEG = -1.0e9

_COMPILED = None


def _body(nc, tc, const, work, ps, pb, d):
    # ---- load constants / inputs into SBUF ----
    xt = []
    for c in range(EC):
        t = const.tile([P, S], BF16, tag=f"xt{c}", name=f"xt{c}")
        nc.sync.dma_start(t[:], d["xt"][c * P:(c + 1) * P, :])
        xt.append(t)
    w_sb = {}
    for wname in ("wq", "wk", "wv"):
        tiles = []
        for c in range(EC):
            t = const.tile([P, HD], BF16, tag=f"{wname}{c}", name=f"{wname}{c}")
            nc.sync.dma_start(t[:], d[wname][c * P:(c + 1) * P, :])
            tiles.append(t)
        w_sb[wname] = tiles
    woh = []
    for h in range(H):
        t = const.tile([DV, E], BF16, tag=f"woh{h}", name=f"woh{h}")
        nc.sync.dma_start(t[:], d["woh"][h])
        woh.append(t)
    mask_t = const.tile([P, 512], F32, tag="mask", name="mask_t")
    nc.sync.dma_start(mask_t[:], d["mask"][:])
    bq_t = const.tile([P, NPAIR], F32, tag="bq", name="bq_t")
    nc.sync.dma_start(bq_t[:], d["bq"][:])
    bk_t = const.tile([P, NPAIR], F32, tag="bk", name="bk_t")
    nc.sync.dma_start(bk_t[:], d["bk"][:])
    bvb_t = const.tile([P, HD], F32, tag="bvb", name="bvb_t")
    nc.sync.dma_start(bvb_t[:], d["bvb"][:])
    bob_t = const.tile([P, E], F32, tag="bob", name="bob_t")
    nc.sync.dma_start(bob_t[:], d["bob"][:])

    # ---- V = X @ Wv + bv : per s-chunk, augmented with a ones column per
    # head ([128 s, 8*65]) so the AV matmul also emits softmax denominators ----
    v_sb = []
    for si in range(S // P):
        vp = ps.tile([P, HD], F32, tag="ps512", name=f"vp{si}")
        for c in range(EC):
            nc.tensor.matmul(
                vp[:], xt[c][:, si * P:(si + 1) * P], w_sb["wv"][c][:],
                start=(c == 0), stop=(c == EC - 1))
        t = const.tile([P, H * 65], BF16, tag=f"v{si}", name=f"v{si}")
        t3 = t.rearrange("p (h c) -> p h c", c=65)
        nc.vector.memset(t[:], 1.0)  # contiguous; leaves the per-head ones column
        nc.vector.tensor_add(
            t3[:, :, 0:DV],
            vp.rearrange("p (h c) -> p h c", c=DV),
            bvb_t.rearrange("p (h c) -> p h c", c=DV))
        v_sb.append(t)

    # ---- QT / KT per head-pair, per q-half: [128 dd, 512 s] ----
    qt = {}
    kt = {}
    for p in range(NPAIR):
        for qc in range(QCN):
            qp = ps.tile([P, 512], F32, tag="ps512", name=f"qtp{p}_{qc}")
            for c in range(EC):
                nc.tensor.matmul(
                    qp[:], w_sb["wq"][c][:, p * P:(p + 1) * P],
                    xt[c][:, qc * 512:(qc + 1) * 512],
                    start=(c == 0), stop=(c == EC - 1))
            t = const.tile([P, 512], BF16, tag=f"qt{p}_{qc}", name=f"qt{p}_{qc}")
            nc.any.tensor_scalar_add(t[:], qp[:], bq_t[:, p:p + 1])
            qt[p, qc] = t

            kp = ps.tile([P, 512], F32, tag="ps512", name=f"ktp{p}_{qc}")
            for c in range(EC):
                nc.tensor.matmul(
                    kp[:], w_sb["wk"][c][:, p * P:(p + 1) * P],
                    xt[c][:, qc * 512:(qc + 1) * 512],
                    start=(c == 0), stop=(c == EC - 1))
            t = const.tile([P, 512], BF16, tag=f"kt{p}_{qc}", name=f"kt{p}_{qc}")
            nc.any.tensor_scalar_add(t[:], kp[:], bk_t[:, p:p + 1])
            kt[p, qc] = t

    # ---- attention per head, per q-chunk ----
    ot_sb = {}
    for h in range(H):
        p, hb = h // 2, h % 2
        hp = slice(hb * DK, (hb + 1) * DK)  # head's rows within pair tiles
        for qc in range(QCN):
            n_ki = 4 * (qc + 1)  # causal: only k-blocks with ki*128 <= qc*512+511
            otp = ps.tile([DV + 1, 512], F32, tag="ps512", name=f"otp{h}_{qc}")
            for g in range(n_ki // 2):
                stp = pb.tile([P, 1024], F32, tag="st", name=f"st{h}_{qc}_{g}")
                for j in range(2):
                    ki = 2 * g + j
                    kc, kl = ki // 4, ki % 4
                    nc.tensor.matmul(
                        stp[:, j * 512:(j + 1) * 512],
                        kt[p, kc][hp, kl * P:(kl + 1) * P],
                        qt[p, qc][hp, :],
                        start=True, stop=True)
                    off = ki * P - qc * 512
                    if off >= 0:
                        w = off + P  # mask region always ends at column 512
                        nc.any.tensor_add(
                            stp[:, j * 512:j * 512 + w],
                            stp[:, j * 512:j * 512 + w],
                            mask_t[:, 512 - w:512])
                ste = work.tile([P, 1024], BF16, tag="ste", name=f"ste{h}_{qc}_{g}")
                nc.scalar.activation(
                    ste[:], stp[:], mybir.ActivationFunctionType.Exp, scale=0.125)
                for j in range(2):
                    ki = 2 * g + j
                    nc.tensor.matmul(
                        otp[:], v_sb[ki][:, h * 65:h * 65 + 65],
                        ste[:, j * 512:(j + 1) * 512],
                        start=(ki == 0), stop=(ki == n_ki - 1))
            rrow = work.tile([1, 512], F32, tag="rrow", name=f"rrow{h}_{qc}", bufs=2)
            nc.vector.tensor_copy(rrow[:], otp[DV:DV + 1, :])
            rec = work.tile([1, 512], F32, tag="rec", name=f"rec{h}_{qc}", bufs=2)
            nc.vector.reciprocal_approx_fast(rec[:], rrow[:])
            rb = work.tile([DV, 512], F32, tag="rb", name=f"rb{h}_{qc}", bufs=2)
            nc.gpsimd.partition_broadcast(rb[:], rec[:])
            ot = const.tile([DV, 512], BF16, tag=f"ot{h}_{qc}", name=f"ot{h}_{qc}")
            nc.vector.tensor_mul(ot[:], otp[0:DV, :], rb[:])
            ot_sb[h, qc] = ot

    # ---- output projection Y[s-chunk] = sum_h OT_h-block^T @ Wo_h + bo ----
    for si in range(S // P):
        qc, sl = si // 4, si % 4
        yp = ps.tile([P, E], F32, tag="ps512", name=f"yp{si}")
        for h in range(H):
            nc.tensor.matmul(
                yp[:], ot_sb[h, qc][:, sl * P:(sl + 1) * P], woh[h][:],
                start=(h == 0), stop=(h == H - 1))
        yo = work.tile([P, E], F32, tag="yo", name=f"yo{si}", bufs=2)
        nc.any.tensor_add(yo[:], yp[:], bob_t[:])
        nc.sync.dma_start(d["y"][si * P:(si + 1) * P, :], yo[:])


def _build():
    nc = bacc.Bacc("TRN2", target_bir_lowering=False, debug=False)
    d = {
        "xt": nc.dram_tensor("xt", [E, S], BF16, kind="ExternalInput").ap(),
        "wq": nc.dram_tensor("wq", [E, HD], BF16, kind="ExternalInput").ap(),
        "wk": nc.dram_tensor("wk", [E, HD], BF16, kind="ExternalInput").ap(),
        "wv": nc.dram_tensor("wv", [E, HD], BF16, kind="ExternalInput").ap(),
        "woh": nc.dram_tensor("woh", [H, DV, E], BF16, kind="ExternalInput").ap(),
        "mask": nc.dram_tensor("mask", [P, 512], F32, kind="ExternalInput").ap(),
        "bq": nc.dram_tensor("bq", [P, NPAIR], F32, kind="ExternalInput").ap(),
        "bk": nc.dram_tensor("bk", [P, NPAIR], F32, kind="ExternalInput").ap(),
        "bvb": nc.dram_tensor("bvb", [P, HD], F32, kind="ExternalInput").ap(),
        "bob": nc.dram_tensor("bob", [P, E], F32, kind="ExternalInput").ap(),
        "y": nc.dram_tensor("y", [S, E], F32, kind="ExternalOutput").ap(),
    }
    with tile.TileContext(nc) as tc:
        with tc.tile_pool(name="const", bufs=1) as const, \
             tc.tile_pool(name="work", bufs=3) as work, \
             tc.tile_pool(name="ps", bufs=4, space="PSUM") as ps, \
             tc.tile_pool(name="pb", bufs=2, space="PSUM") as pb:
            _body(nc, tc, const, work, ps, pb, d)
    nc.compile()
    return nc


def get_nc():
    global _COMPILED
    if _COMPILED is None:
        _COMPILED = _build()
    return _COMPILED


def _prep_in_maps(X, Wq, bq, Wk, bk, Wv, bv, Wo, bo):
    f = np.float32
    bf = ml_dtypes.bfloat16
    shared = {
        "wq": np.ascontiguousarray(
            np.transpose(np.asarray(Wq, f), (1, 0, 2)).reshape(E, HD).astype(bf)),
        "wk": np.ascontiguousarray(
            np.transpose(np.asarray(Wk, f), (1, 0, 2)).reshape(E, HD).astype(bf)),
        "wv": np.ascontiguousarray(
            np.transpose(np.asarray(Wv, f), (1, 0, 2)).reshape(E, HD).astype(bf)),
        "woh": np.ascontiguousarray(np.asarray(Wo, f).reshape(H, DV, E).astype(bf)),
        "bq": np.ascontiguousarray(np.asarray(bq, f).reshape(HD).reshape(NPAIR, P).T),
        "bk": np.ascontiguousarray(np.asarray(bk, f).reshape(HD).reshape(NPAIR, P).T),
        "bvb": np.ascontiguousarray(np.broadcast_to(np.asarray(bv, f).reshape(1, HD), (P, HD))),
        "bob": np.ascontiguousarray(np.broadcast_to(np.asarray(bo, f).reshape(1, E), (P, E))),
    }
    # staircase causal mask slab: M[k, j] = NEG where k > j - 384 (j in [0,512)).
    # block (ki, qc) with off = ki*128 - qc*512 >= 0 uses columns [512-w, 512).
    kk = np.arange(P)[:, None]
    jj = np.arange(512)[None, :]
    shared["mask"] = np.where(kk > jj - 384, f(NEG), f(0.0)).astype(f)
    Xf = np.asarray(X, f)
    in_maps = []
    for b in range(B):
        m = dict(shared)
        m["xt"] = np.ascontiguousarray(Xf[b].T.astype(bf))
        in_maps.append(m)
    return in_maps


def kernel(X, Wq, bq, Wk, bk, Wv, bv, Wo, bo):
    nc = get_nc()
    in_maps = _prep_in_maps(X, Wq, bq, Wk, bk, Wv, bv, Wo, bo)
    res = bass_utils.run_bass_kernel_spmd(nc, in_maps, core_ids=list(range(NCORES)))
    return np.stack([res.results[b]["y"] for b in range(B)], axis=0).astype(np.float32)


def run_traced(X, Wq, bq, Wk, bk, Wv, bv, Wo, bo):
    """Like kernel() but with NTFF profiling; returns (out, exec_time_ns)."""
    nc = get_nc()
    in_maps = _prep_in_maps(X, Wq, bq, Wk, bk, Wv, bv, Wo, bo)
    res = bass_utils.run_bass_kernel_spmd(
        nc, in_maps, core_ids=list(range(NCORES)), trace=True)
    out = np.stack([res.results[b]["y"] for b in range(B)], axis=0).astype(np.float32)
    return out, res.exec_time_ns


# revision 11
# speedup vs baseline: 1.7531x; 1.0351x over previous
"""Multi-head causal attention (B=8, S=1024, E=512, H=8, Dk=Dv=64) on 8 NeuronCores.

Sharding: data-parallel over batch. Core b computes the full attention block
for X[b]; no collectives. Host pre-transposes X[b] -> [E, S], converts matmul
operands to bf16, and pre-arranges weights so the device kernel is pure
matmul + softmax.

Per-core dataflow (bf16 matmuls, fp32 PSUM accumulate / softmax math):
  XT [E,S] resident in SBUF (4 tiles of [128, 1024])
  V  = (X @ Wv + bv)   -> 8 tiles [128 s, 8*65], each head's 64 V columns
                          augmented with a ones column so the AV matmul also
                          emits the softmax denominator row (M=65)
  QT = (X @ Wq + bq)^T -> per head-pair tiles [128 dd, 512 q] (x2 q-halves)
  KT likewise
  per head-pair p, q-chunk qc (512 cols):
    the two heads' score^T blocks [128 k, 512 q] are emitted back-to-back on
    disjoint PE row halves (rows 0-63 / 64-127 via base-partition row tiling)
    so they overlap in the systolic array; k-blocks above the causal diagonal
    are skipped and fully-masked columns [0, off) are never computed;
    the diagonal 128x128 triangle gets a -1e9 additive mask; exp on ScalarE
    (scale=1/8 folded in) reads both heads' trimmed regions in one 3D-AP op;
    O^T/denom accumulate via the augmented-V matmul (M=65, trimmed cols);
    O^T *= 1/denom (fast-NR reciprocal on DVE + gpsimd partition broadcast).
  Y[s-chunk] = sum_h O_h^T-block^T @ Wo_h + bo
"""

import numpy as np
import ml_dtypes

import concourse.bass as bass
import concourse.tile as tile
import concourse.mybir as mybir
from concourse import bacc
from concourse import bass_utils

B, S, E = 8, 1024, 512
H, DK, DV = 8, 64, 64
HD = H * DK  # 512
P = 128
EC = E // P  # 4 contraction chunks over E
NPAIR = H // 2
QCN = S // 512  # 2 q-chunks of 512
NCORES = 8
F32 = mybir.dt.float32
BF16 = mybir.dt.bfloat16
NEG = -1.0e9

_COMPILED = None


def _body(nc, tc, const, work, ps, pb, d):
    # ---- load constants / inputs into SBUF ----
    xt = []
    for c in range(EC):
        t = const.tile([P, S], BF16, tag=f"xt{c}", name=f"xt{c}")
        nc.sync.dma_start(t[:], d["xt"][c * P:(c + 1) * P, :])
        xt.append(t)
    w_sb = {}
    for wname in ("wq", "wk", "wv"):
        tiles = []
        for c in range(EC):
            t = const.tile([P, HD], BF16, tag=f"{wname}{c}", name=f"{wname}{c}")
            nc.sync.dma_start(t[:], d[wname][c * P:(c + 1) * P, :])
            tiles.append(t)
        w_sb[wname] = tiles
    woh = []
    for h in range(H):
        t = const.tile([DV, E], BF16, tag=f"woh{h}", name=f"woh{h}")
        nc.sync.dma_start(t[:], d["woh"][h])
        woh.append(t)
    mask_t = const.tile([P, 512], F32, tag="mask", name="mask_t")
    nc.sync.dma_start(mask_t[:], d["mask"][:])
    bq_t = const.tile([P, NPAIR], F32, tag="bq", name="bq_t")
    nc.sync.dma_start(bq_t[:], d["bq"][:])
    bk_t = const.tile([P, NPAIR], F32, tag="bk", name="bk_t")
    nc.sync.dma_start(bk_t[:], d["bk"][:])
    bvb_t = const.tile([P, HD], F32, tag="bvb", name="bvb_t")
    nc.sync.dma_start(bvb_t[:], d["bvb"][:])
    bob_t = const.tile([P, E], F32, tag="bob", name="bob_t")
    nc.sync.dma_start(bob_t[:], d["bob"][:])

    # ---- V = X @ Wv + bv : per s-chunk, augmented with a ones column per
    # head ([128 s, 8*65]) so the AV matmul also emits softmax denominators ----
    v_sb = []
    for si in range(S // P):
        vp = ps.tile([P, HD], F32, tag="ps512", name=f"vp{si}")
        for c in range(EC):
            nc.tensor.matmul(
                vp[:], xt[c][:, si * P:(si + 1) * P], w_sb["wv"][c][:],
                start=(c == 0), stop=(c == EC - 1))
        t = const.tile([P, H * 65], BF16, tag=f"v{si}", name=f"v{si}")
        t3 = t.rearrange("p (h c) -> p h c", c=65)
        nc.vector.memset(t[:], 1.0)  # contiguous; leaves the per-head ones column
        nc.vector.tensor_add(
            t3[:, :, 0:DV],
            vp.rearrange("p (h c) -> p h c", c=DV),
            bvb_t.rearrange("p (h c) -> p h c", c=DV))
        v_sb.append(t)

    # ---- QT / KT per head-pair, per q-half: [128 dd, 512 s] ----
    qt = {}
    kt = {}
    for p in range(NPAIR):
        for qc in range(QCN):
            qp = ps.tile([P, 512], F32, tag="ps512", name=f"qtp{p}_{qc}")
            for c in range(EC):
                nc.tensor.matmul(
                    qp[:], w_sb["wq"][c][:, p * P:(p + 1) * P],
                    xt[c][:, qc * 512:(qc + 1) * 512],
                    start=(c == 0), stop=(c == EC - 1))
            t = const.tile([P, 512], BF16, tag=f"qt{p}_{qc}", name=f"qt{p}_{qc}")
            nc.any.tensor_scalar_add(t[:], qp[:], bq_t[:, p:p + 1])
            qt[p, qc] = t

            kp = ps.tile([P, 512], F32, tag="ps512", name=f"ktp{p}_{qc}")
            for c in range(EC):
                nc.tensor.matmul(
                    kp[:], w_sb["wk"][c][:, p * P:(p + 1) * P],
                    xt[c][:, qc * 512:(qc + 1) * 512],
                    start=(c == 0), stop=(c == EC - 1))
            t = const.tile([P, 512], BF16, tag=f"kt{p}_{qc}", name=f"kt{p}_{qc}")
            nc.any.tensor_scalar_add(t[:], kp[:], bk_t[:, p:p + 1])
            kt[p, qc] = t

    # ---- attention per head-pair, per q-chunk.  The two heads' score
    # matmuls are emitted back-to-back on disjoint PE row halves (rows 0-63 /
    # 64-127 via base-partition row tiling) so they overlap in the array.
    # Causal trimming: fully-masked columns [0, off) of a k-block are never
    # computed, exp'd, or consumed. ----
    ot_sb = {}
    for p in range(NPAIR):
        for qc in range(QCN):
            n_ki = 4 * (qc + 1)  # causal: only k-blocks with ki*128 <= qc*512+511
            otp = {}
            for hb in range(2):
                otp[hb] = ps.tile([DV + 1, 512], F32, tag="ps512",
                                  name=f"otp{p}_{qc}_{hb}")
            for ki in range(n_ki):
                kc, kl = ki // 4, ki % 4
                diag = (ki * P - qc * 512) >= 0
                off = max(ki * P - qc * 512, 0)
                stp = pb.tile([P, 1024], F32, tag="st", name=f"st{p}_{qc}_{ki}")
                for hb in range(2):
                    hp = slice(hb * DK, (hb + 1) * DK)
                    nc.tensor.matmul(
                        stp[:, hb * 512 + off:(hb + 1) * 512],
                        kt[p, kc][hp, kl * P:(kl + 1) * P],
                        qt[p, qc][hp, off:],
                        start=True, stop=True)
                if diag:
                    # triangle mask on the diagonal 128 columns of both halves
                    for hb in range(2):
                        nc.any.tensor_add(
                            stp[:, hb * 512 + off:hb * 512 + off + P],
                            stp[:, hb * 512 + off:hb * 512 + off + P],
                            mask_t[:, 384:512])
                ste = work.tile([P, 1024], BF16, tag="ste", name=f"ste{p}_{qc}_{ki}")
                stp3 = stp.rearrange("p (h q) -> p h q", h=2)[:, :, off:]
                ste3 = ste.rearrange("p (h q) -> p h q", h=2)[:, :, off:]
                nc.scalar.activation(
                    ste3, stp3, mybir.ActivationFunctionType.Exp, scale=0.125)
                st_f, sp_f = (ki == 0), (ki == n_ki - 1)
                for hb in range(2):
                    h = 2 * p + hb
                    nc.tensor.matmul(
                        otp[hb][:, off:], v_sb[ki][:, h * 65:h * 65 + 65],
                        ste[:, hb * 512 + off:(hb + 1) * 512],
                        start=st_f, stop=sp_f, skip_group_check=True)
            for hb in range(2):
                h = 2 * p + hb
                rrow = work.tile([1, 512], F32, tag="rrow", name=f"rrow{h}_{qc}", bufs=2)
                nc.vector.tensor_copy(rrow[:], otp[hb][DV:DV + 1, :])
                rec = work.tile([1, 512], F32, tag="rec", name=f"rec{h}_{qc}", bufs=2)
                nc.vector.reciprocal_approx_fast(rec[:], rrow[:])
                rb = work.tile([DV, 512], F32, tag="rb", name=f"rb{h}_{qc}", bufs=2)
                nc.gpsimd.partition_broadcast(rb[:], rec[:])
                ot = const.tile([DV, 512], BF16, tag=f"ot{h}_{qc}", name=f"ot{h}_{qc}")
                nc.vector.tensor_mul(ot[:], otp[hb][0:DV, :], rb[:])
                ot_sb[h, qc] = ot

    # ---- output projection Y[s-chunk] = sum_h OT_h-block^T @ Wo_h + bo ----
    for si in range(S // P):
        qc, sl = si // 4, si % 4
        yp = ps.tile([P, E], F32, tag="ps512", name=f"yp{si}")
        for h in range(H):
            nc.tensor.matmul(
                yp[:], ot_sb[h, qc][:, sl * P:(sl + 1) * P], woh[h][:],
                start=(h == 0), stop=(h == H - 1))
        yo = work.tile([P, E], F32, tag="yo", name=f"yo{si}", bufs=2)
        nc.any.tensor_add(yo[:], yp[:], bob_t[:])
        nc.sync.dma_start(d["y"][si * P:(si + 1) * P, :], yo[:])


def _build():
    nc = bacc.Bacc("TRN2", target_bir_lowering=False, debug=False)
    d = {
        "xt": nc.dram_tensor("xt", [E, S], BF16, kind="ExternalInput").ap(),
        "wq": nc.dram_tensor("wq", [E, HD], BF16, kind="ExternalInput").ap(),
        "wk": nc.dram_tensor("wk", [E, HD], BF16, kind="ExternalInput").ap(),
        "wv": nc.dram_tensor("wv", [E, HD], BF16, kind="ExternalInput").ap(),
        "woh": nc.dram_tensor("woh", [H, DV, E], BF16, kind="ExternalInput").ap(),
        "mask": nc.dram_tensor("mask", [P, 512], F32, kind="ExternalInput").ap(),
        "bq": nc.dram_tensor("bq", [P, NPAIR], F32, kind="ExternalInput").ap(),
        "bk": nc.dram_tensor("bk", [P, NPAIR], F32, kind="ExternalInput").ap(),
        "bvb": nc.dram_tensor("bvb", [P, HD], F32, kind="ExternalInput").ap(),
        "bob": nc.dram_tensor("bob", [P, E], F32, kind="ExternalInput").ap(),
        "y": nc.dram_tensor("y", [S, E], F32, kind="ExternalOutput").ap(),
    }
    with tile.TileContext(nc) as tc:
        with tc.tile_pool(name="const", bufs=1) as const, \
             tc.tile_pool(name="work", bufs=3) as work, \
             tc.tile_pool(name="ps", bufs=4, space="PSUM") as ps, \
             tc.tile_pool(name="pb", bufs=2, space="PSUM") as pb:
            _body(nc, tc, const, work, ps, pb, d)
    nc.compile()
    return nc


def get_nc():
    global _COMPILED
    if _COMPILED is None:
        _COMPILED = _build()
    return _COMPILED


def _prep_in_maps(X, Wq, bq, Wk, bk, Wv, bv, Wo, bo):
    f = np.float32
    bf = ml_dtypes.bfloat16
    shared = {
        "wq": np.ascontiguousarray(
            np.transpose(np.asarray(Wq, f), (1, 0, 2)).reshape(E, HD).astype(bf)),
        "wk": np.ascontiguousarray(
            np.transpose(np.asarray(Wk, f), (1, 0, 2)).reshape(E, HD).astype(bf)),
        "wv": np.ascontiguousarray(
            np.transpose(np.asarray(Wv, f), (1, 0, 2)).reshape(E, HD).astype(bf)),
        "woh": np.ascontiguousarray(np.asarray(Wo, f).reshape(H, DV, E).astype(bf)),
        "bq": np.ascontiguousarray(np.asarray(bq, f).reshape(HD).reshape(NPAIR, P).T),
        "bk": np.ascontiguousarray(np.asarray(bk, f).reshape(HD).reshape(NPAIR, P).T),
        "bvb": np.ascontiguousarray(np.broadcast_to(np.asarray(bv, f).reshape(1, HD), (P, HD))),
        "bob": np.ascontiguousarray(np.broadcast_to(np.asarray(bo, f).reshape(1, E), (P, E))),
    }
    # staircase causal mask slab: M[k, j] = NEG where k > j - 384 (j in [0,512)).
    # block (ki, qc) with off = ki*128 - qc*512 >= 0 uses columns [512-w, 512).
    kk = np.arange(P)[:, None]
    jj = np.arange(512)[None, :]
    shared["mask"] = np.where(kk > jj - 384, f(NEG), f(0.0)).astype(f)
    Xf = np.asarray(X, f)
    in_maps = []
    for b in range(B):
        m = dict(shared)
        m["xt"] = np.ascontiguousarray(Xf[b].T.astype(bf))
        in_maps.append(m)
    return in_maps


def kernel(X, Wq, bq, Wk, bk, Wv, bv, Wo, bo):
    nc = get_nc()
    in_maps = _prep_in_maps(X, Wq, bq, Wk, bk, Wv, bv, Wo, bo)
    last_exc = None
    for attempt in range(3):
        try:
            res = bass_utils.run_bass_kernel_spmd(
                nc, in_maps, core_ids=list(range(NCORES)))
            break
        except Exception as e:  # sporadic NRT_EXEC_UNIT_UNRECOVERABLE on first exec
            last_exc = e
            import time
            time.sleep(15)
    else:
        raise last_exc
    return np.stack([res.results[b]["y"] for b in range(B)], axis=0).astype(np.float32)


def run_traced(X, Wq, bq, Wk, bk, Wv, bv, Wo, bo):
    """Like kernel() but with NTFF profiling; returns (out, exec_time_ns)."""
    nc = get_nc()
    in_maps = _prep_in_maps(X, Wq, bq, Wk, bk, Wv, bv, Wo, bo)
    res = bass_utils.run_bass_kernel_spmd(
        nc, in_maps, core_ids=list(range(NCORES)), trace=True)
    out = np.stack([res.results[b]["y"] for b in range(B)], axis=0).astype(np.float32)
    return out, res.exec_time_ns


# revision 12
# speedup vs baseline: 1.8730x; 1.0684x over previous
"""Multi-head causal attention (B=8, S=1024, E=512, H=8, Dk=Dv=64) on 8 NeuronCores.

Sharding: data-parallel over batch. Core b computes the full attention block
for X[b]; no collectives. Host pre-transposes X[b] -> [E, S], converts matmul
operands to bf16, and pre-arranges weights so the device kernel is pure
matmul + softmax.

Per-core dataflow (bf16 matmuls, fp32 PSUM accumulate / softmax math):
  XT [E,S] resident in SBUF (4 tiles of [128, 1024])
  V  = (X @ Wv + bv)            -> 8 tiles [128 s, 512 hd]
  QT = (X @ Wq + bq)^T          -> per head-pair tiles [128 dd, 512 q] (x2 q-halves)
  KT likewise
  per head h, q-chunk qc (512 cols):
    scores^T blocks [128 k, 512 q] (k-blocks above the diagonal skipped),
    causal mask added from a precomputed staircase slab, exp on ScalarE
    (scale=1/8 folded in) over two-block PSUM groups,
    O^T accum = V-slice^T @ exp-blocks,  denom = ones^T @ exp-blocks
    (ones lhsT is M=128 so the denominator row broadcasts),
    O^T *= 1/denom via fast-NR reciprocal.
  Y[s-chunk] = sum_h O_h^T-block^T @ Wo_h + bo
"""

import numpy as np
import ml_dtypes

import concourse.bass as bass
import concourse.tile as tile
import concourse.mybir as mybir
from concourse import bacc
from concourse import bass_utils

B, S, E = 8, 1024, 512
H, DK, DV = 8, 64, 64
HD = H * DK  # 512
P = 128
EC = E // P  # 4 contraction chunks over E
NPAIR = H // 2
QCN = S // 512  # 2 q-chunks of 512
NCORES = 8
F32 = mybir.dt.float32
BF16 = mybir.dt.bfloat16
NEG = -1.0e9

_COMPILED = None


def _body(nc, tc, const, work, ps, pb, d):
    # ---- load constants / inputs into SBUF ----
    xt = []
    for c in range(EC):
        t = const.tile([P, S], BF16, tag=f"xt{c}", name=f"xt{c}")
        nc.sync.dma_start(t[:], d["xt"][c * P:(c + 1) * P, :])
        xt.append(t)
    w_sb = {}
    for wname in ("wq", "wk", "wv"):
        tiles = []
        for c in range(EC):
            t = const.tile([P, HD], BF16, tag=f"{wname}{c}", name=f"{wname}{c}")
            nc.sync.dma_start(t[:], d[wname][c * P:(c + 1) * P, :])
            tiles.append(t)
        w_sb[wname] = tiles
    w_sb["wo"] = []
    for c in range(EC):
        t = const.tile([P, E], BF16, tag=f"wo{c}", name=f"wo{c}")
        nc.sync.dma_start(t[:], d["wo"][c * P:(c + 1) * P, :])
        w_sb["wo"].append(t)
    mask_t = const.tile([P, 512], F32, tag="mask", name="mask_t")
    nc.sync.dma_start(mask_t[:], d["mask"][:])
    bq_t = const.tile([P, NPAIR], F32, tag="bq", name="bq_t")
    nc.sync.dma_start(bq_t[:], d["bq"][:])
    bk_t = const.tile([P, NPAIR], F32, tag="bk", name="bk_t")
    nc.sync.dma_start(bk_t[:], d["bk"][:])
    bvb_t = const.tile([P, HD], F32, tag="bvb", name="bvb_t")
    nc.sync.dma_start(bvb_t[:], d["bvb"][:])
    bob_t = const.tile([P, E], F32, tag="bob", name="bob_t")
    nc.sync.dma_start(bob_t[:], d["bob"][:])

    # ---- V = X @ Wv + bv : per s-chunk, augmented with a ones column per
    # head ([128 s, 8*65]) so the AV matmul also emits softmax denominators ----
    v_sb = []
    for si in range(S // P):
        vp = ps.tile([P, HD], F32, tag="ps512", name=f"vp{si}")
        for c in range(EC):
            nc.tensor.matmul(
                vp[:], xt[c][:, si * P:(si + 1) * P], w_sb["wv"][c][:],
                start=(c == 0), stop=(c == EC - 1))
        t = const.tile([P, H * 65], BF16, tag=f"v{si}", name=f"v{si}")
        t3 = t.rearrange("p (h c) -> p h c", c=65)
        nc.vector.memset(t[:], 1.0)  # contiguous; leaves the per-head ones column
        nc.vector.tensor_add(
            t3[:, :, 0:DV],
            vp.rearrange("p (h c) -> p h c", c=DV),
            bvb_t.rearrange("p (h c) -> p h c", c=DV))
        v_sb.append(t)

    # ---- QT / KT per head-pair, per q-half: [128 dd, 512 s] ----
    qt = {}
    kt = {}
    for p in range(NPAIR):
        for qc in range(QCN):
            qp = ps.tile([P, 512], F32, tag="ps512", name=f"qtp{p}_{qc}")
            for c in range(EC):
                nc.tensor.matmul(
                    qp[:], w_sb["wq"][c][:, p * P:(p + 1) * P],
                    xt[c][:, qc * 512:(qc + 1) * 512],
                    start=(c == 0), stop=(c == EC - 1))
            t = const.tile([P, 512], BF16, tag=f"qt{p}_{qc}", name=f"qt{p}_{qc}")
            nc.any.tensor_scalar_add(t[:], qp[:], bq_t[:, p:p + 1])
            qt[p, qc] = t

            kp = ps.tile([P, 512], F32, tag="ps512", name=f"ktp{p}_{qc}")
            for c in range(EC):
                nc.tensor.matmul(
                    kp[:], w_sb["wk"][c][:, p * P:(p + 1) * P],
                    xt[c][:, qc * 512:(qc + 1) * 512],
                    start=(c == 0), stop=(c == EC - 1))
            t = const.tile([P, 512], BF16, tag=f"kt{p}_{qc}", name=f"kt{p}_{qc}")
            nc.any.tensor_scalar_add(t[:], kp[:], bk_t[:, p:p + 1])
            kt[p, qc] = t

    # ---- attention per head-pair, per q-chunk.  The two heads' score
    # matmuls are emitted back-to-back on disjoint PE row halves (rows 0-63 /
    # 64-127 via base-partition row tiling) so they overlap in the array.
    # Causal trimming: fully-masked columns [0, off) of a k-block are never
    # computed, exp'd, or consumed. ----
    ot_sb = {}
    for p in range(NPAIR):
        for qc in range(QCN):
            n_ki = 4 * (qc + 1)  # causal: only k-blocks with ki*128 <= qc*512+511
            otp = {}
            for hb in range(2):
                otp[hb] = ps.tile([DV + 1, 512], F32, tag="ps512",
                                  name=f"otp{p}_{qc}_{hb}")
            for ki in range(n_ki):
                kc, kl = ki // 4, ki % 4
                diag = (ki * P - qc * 512) >= 0
                off = max(ki * P - qc * 512, 0)
                stp = pb.tile([P, 1024], F32, tag="st", name=f"st{p}_{qc}_{ki}")
                for hb in range(2):
                    hp = slice(hb * DK, (hb + 1) * DK)
                    nc.tensor.matmul(
                        stp[:, hb * 512 + off:(hb + 1) * 512],
                        kt[p, kc][hp, kl * P:(kl + 1) * P],
                        qt[p, qc][hp, off:],
                        start=True, stop=True, tile_position=(hb * DK, 0))
                if diag:
                    # triangle mask on the diagonal 128 columns of both halves
                    for hb in range(2):
                        nc.any.tensor_add(
                            stp[:, hb * 512 + off:hb * 512 + off + P],
                            stp[:, hb * 512 + off:hb * 512 + off + P],
                            mask_t[:, 384:512])
                ste = work.tile([P, 1024], BF16, tag="ste", name=f"ste{p}_{qc}_{ki}")
                stp3 = stp.rearrange("p (h q) -> p h q", h=2)[:, :, off:]
                ste3 = ste.rearrange("p (h q) -> p h q", h=2)[:, :, off:]
                nc.scalar.activation(
                    ste3, stp3, mybir.ActivationFunctionType.Exp, scale=0.125)
                st_f, sp_f = (ki == 0), (ki == n_ki - 1)
                for hb in range(2):
                    h = 2 * p + hb
                    nc.tensor.matmul(
                        otp[hb][:, off:], v_sb[ki][:, h * 65:h * 65 + 65],
                        ste[:, hb * 512 + off:(hb + 1) * 512],
                        start=st_f, stop=sp_f, skip_group_check=True)
            ot = const.tile([P, 512], BF16, tag=f"ot{p}_{qc}", name=f"ot{p}_{qc}")
            for hb in range(2):
                h = 2 * p + hb
                rrow = work.tile([1, 512], F32, tag="rrow", name=f"rrow{h}_{qc}", bufs=2)
                nc.vector.tensor_copy(rrow[:], otp[hb][DV:DV + 1, :])
                rec = work.tile([1, 512], F32, tag="rec", name=f"rec{h}_{qc}", bufs=2)
                nc.vector.reciprocal_approx_fast(rec[:], rrow[:])
                rb = work.tile([DV, 512], F32, tag="rb", name=f"rb{h}_{qc}", bufs=2)
                nc.gpsimd.partition_broadcast(rb[:], rec[:])
                if hb == 0:
                    nc.vector.tensor_mul(ot[0:DV, :], otp[0][0:DV, :], rb[:])
                else:
                    # DVE cannot shift partitions: scale into a temp at base 0,
                    # then SBUF->SBUF DMA into partitions 64-127 of the pair tile
                    tmp = work.tile([DV, 512], BF16, tag="ottmp",
                                    name=f"ottmp{p}_{qc}", bufs=2)
                    nc.vector.tensor_mul(tmp[:], otp[1][0:DV, :], rb[:])
                    nc.sync.dma_start(ot[DV:P, :], tmp[:])
            ot_sb[p, qc] = ot

    # ---- output projection Y[s-chunk] = sum_p OT_pair-block^T @ Wo-chunk + bo ----
    for si in range(S // P):
        qc, sl = si // 4, si % 4
        yp = ps.tile([P, E], F32, tag="ps512", name=f"yp{si}")
        for p in range(NPAIR):
            nc.tensor.matmul(
                yp[:], ot_sb[p, qc][:, sl * P:(sl + 1) * P], w_sb["wo"][p][:],
                start=(p == 0), stop=(p == NPAIR - 1))
        yo = work.tile([P, E], F32, tag="yo", name=f"yo{si}", bufs=2)
        nc.any.tensor_add(yo[:], yp[:], bob_t[:])
        nc.sync.dma_start(d["y"][si * P:(si + 1) * P, :], yo[:])


def _build():
    nc = bacc.Bacc("TRN2", target_bir_lowering=False, debug=False)
    d = {
        "xt": nc.dram_tensor("xt", [E, S], BF16, kind="ExternalInput").ap(),
        "wq": nc.dram_tensor("wq", [E, HD], BF16, kind="ExternalInput").ap(),
        "wk": nc.dram_tensor("wk", [E, HD], BF16, kind="ExternalInput").ap(),
        "wv": nc.dram_tensor("wv", [E, HD], BF16, kind="ExternalInput").ap(),
        "wo": nc.dram_tensor("wo", [HD, E], BF16, kind="ExternalInput").ap(),
        "mask": nc.dram_tensor("mask", [P, 512], F32, kind="ExternalInput").ap(),
        "bq": nc.dram_tensor("bq", [P, NPAIR], F32, kind="ExternalInput").ap(),
        "bk": nc.dram_tensor("bk", [P, NPAIR], F32, kind="ExternalInput").ap(),
        "bvb": nc.dram_tensor("bvb", [P, HD], F32, kind="ExternalInput").ap(),
        "bob": nc.dram_tensor("bob", [P, E], F32, kind="ExternalInput").ap(),
        "y": nc.dram_tensor("y", [S, E], F32, kind="ExternalOutput").ap(),
    }
    with tile.TileContext(nc) as tc:
        with tc.tile_pool(name="const", bufs=1) as const, \
             tc.tile_pool(name="work", bufs=3) as work, \
             tc.tile_pool(name="ps", bufs=4, space="PSUM") as ps, \
             tc.tile_pool(name="pb", bufs=2, space="PSUM") as pb:
            _body(nc, tc, const, work, ps, pb, d)
    nc.compile()
    return nc


def get_nc():
    global _COMPILED
    if _COMPILED is None:
        _COMPILED = _build()
    return _COMPILED


def _prep_in_maps(X, Wq, bq, Wk, bk, Wv, bv, Wo, bo):
    f = np.float32
    bf = ml_dtypes.bfloat16
    shared = {
        "wq": np.ascontiguousarray(
            np.transpose(np.asarray(Wq, f), (1, 0, 2)).reshape(E, HD).astype(bf)),
        "wk": np.ascontiguousarray(
            np.transpose(np.asarray(Wk, f), (1, 0, 2)).reshape(E, HD).astype(bf)),
        "wv": np.ascontiguousarray(
            np.transpose(np.asarray(Wv, f), (1, 0, 2)).reshape(E, HD).astype(bf)),
        "wo": np.ascontiguousarray(np.asarray(Wo, f).reshape(HD, E).astype(bf)),
        "bq": np.ascontiguousarray(np.asarray(bq, f).reshape(HD).reshape(NPAIR, P).T),
        "bk": np.ascontiguousarray(np.asarray(bk, f).reshape(HD).reshape(NPAIR, P).T),
        "bvb": np.ascontiguousarray(np.broadcast_to(np.asarray(bv, f).reshape(1, HD), (P, HD))),
        "bob": np.ascontiguousarray(np.broadcast_to(np.asarray(bo, f).reshape(1, E), (P, E))),
    }
    # staircase causal mask slab: M[k, j] = NEG where k > j - 384 (j in [0,512)).
    # block (ki, qc) with off = ki*128 - qc*512 >= 0 uses columns [512-w, 512).
    kk = np.arange(P)[:, None]
    jj = np.arange(512)[None, :]
    shared["mask"] = np.where(kk > jj - 384, f(NEG), f(0.0)).astype(f)
    Xf = np.asarray(X, f)
    in_maps = []
    for b in range(B):
        m = dict(shared)
        m["xt"] = np.ascontiguousarray(Xf[b].T.astype(bf))
        in_maps.append(m)
    return in_maps


def kernel(X, Wq, bq, Wk, bk, Wv, bv, Wo, bo):
    nc = get_nc()
    in_maps = _prep_in_maps(X, Wq, bq, Wk, bk, Wv, bv, Wo, bo)
    res = bass_utils.run_bass_kernel_spmd(nc, in_maps, core_ids=list(range(NCORES)))
    return np.stack([res.results[b]["y"] for b in range(B)], axis=0).astype(np.float32)


def run_traced(X, Wq, bq, Wk, bk, Wv, bv, Wo, bo):
    """Like kernel() but with NTFF profiling; returns (out, exec_time_ns)."""
    nc = get_nc()
    in_maps = _prep_in_maps(X, Wq, bq, Wk, bk, Wv, bv, Wo, bo)
    res = bass_utils.run_bass_kernel_spmd(
        nc, in_maps, core_ids=list(range(NCORES)), trace=True)
    out = np.stack([res.results[b]["y"] for b in range(B)], axis=0).astype(np.float32)
    return out, res.exec_time_ns
